# revision 111
# baseline (speedup 1.0000x reference)
"""Trainium2 Bass kernel for nn_FComb_79319456023150 (dense_cnn).

Per-pixel MLP over a 96^3 volume: four 1x1x1 convs (38->32->32->32->1 channels,
relu between). z is batch-constant, so w1[:, 32:38] @ z folds into the layer-1
bias and every layer becomes a K=32 channel GEMM.

Sharding: spatial (outermost X axis) across 8 cores, 110592 pixels each;
weights/biases replicated. Host restripes each shard to [128, 27648] bf16 =
4 pixel-blocks x 32 channels on partitions, pixels on the free dim (bf16
halves input DMA bytes; rel err ~5e-3 vs the 2e-2 tolerance).

Layers 1-3 use a block-diagonal [128,128] bf16 weight (4 copies of W^T), one
matmul per 512-col PSUM bank chunk, so one full-partition matmul applies the
32x32 GEMM to 4 pixel blocks at once. Schedule: depth-4 software pipeline --
tick t emits matmuls for L1(t), L2(t-1), L3(t-2), L4(t-3) -- so each relu
crossing's ~1.3us latency hides behind three other super-chunks' matmuls.

PSUM is ONE 8-bank tile with subtile range deps. Banks 0-6 rotate data
chunks (adjacent pairs, reuse distance ~2 ticks >> crossing latency); pairs
that wrap bank 6->0 split into two half-crossings so every engine/DMA AP
keeps its partition range in the first AP dim (SBUF APs address partitions
ONLY via dim 0 -- nested partition splits silently read garbage). Bank 7 is
a stationary L4 accumulator: sc s writes rows 32q..32q+7 (q = s mod 4, via
per-chunk [128,8] lhsT + tile_position=(0,32q)), so ONE [128,512] evac op
covers 4 super-chunks -- cutting the L4 evacuation engine work ~2.5x vs
per-sc evacuation.

The relu+bias crossings (fp32 PSUM reads run 1 elem/cycle/lane; GPSIMD has
no PSUM port) are greedily balanced between ScalarE (1.2 GHz) and VectorE
(0.96 GHz) by accumulated busy time -- these two engines are the throughput
bound (~49.5us each). Output DMAs (2 per sc, [4,512] plain partition
slices) split their ~0.65us/DMA issue cost between the SP and idle Pool
sequencers; the drain tail is SP-only. Startup: the first input super-chunk
rides Pool's SWDGE in parallel with the w1+L4+bias-bits DMA on SP, and
dummy matmuls on a memset scratch tile burn the PE p-state ramp during the
DMA wait. 64777 ns vs the 71309 ns baseline (cost-model timeline).
"""

import sys

import numpy as np

if "/opt/trn_rl_repo" not in sys.path:
    sys.path.insert(0, "/opt/trn_rl_repo")

C = 32          # channels per layer
P = 128         # SBUF/PSUM partitions
RG = 4          # pixel blocks stacked on the partition dim (128/32)
NCHUNK = 2      # 512-wide chunks per super-chunk (PSUM slot = 2 banks)
CH = 512        # chunk width (one PSUM bank of fp32)
SCW = NCHUNK * CH                    # 1024 free-dim columns per super-chunk
VOL = 96 * 96 * 96                   # full volume
NCORES = 8
NPIX = VOL // NCORES                 # 110592 pixels per core
FREE = NPIX // RG                    # 27648 free-dim columns per core
NSC = FREE // SCW                    # 27 super-chunks per core
OROWS = RG * NCHUNK                  # 8 packed output rows per super-chunk
assert FREE % SCW == 0

N_WARMUP = 3                         # dummy PE matmuls during startup DMA
IN_GROUPS = [1, 1, 1] + [3] * 8      # super-chunks per input DMA
PREFETCH = 5                         # emit input DMA this many ticks early
L4_GROUPS = [4] * 6 + [1, 1, 1]      # super-chunks per packed L4 bank epoch
assert sum(IN_GROUPS) == NSC and sum(L4_GROUPS) == NSC

# engine busy-time estimates (ns) for the greedy balancer. A fin4 op is
# [128, 512]: engine time depends only on the 512-col free size, so it
# costs the same as a half crossing.
COST = {
    ("A", "big"): 1038.0, ("D", "big"): 1192.0,
    ("A", "fin4"): 612.0, ("D", "fin4"): 658.0,
    ("A", "half"): 612.0, ("D", "half"): 658.0,
}
L4GROUP = 4                          # super-chunks packed per L4 PSUM bank
NDB = 7                              # data banks in the chunk rotation


def _build_nc():
    import concourse.mybir as mybir
    from concourse import bacc
    from concourse.tile import TileContext
    from concourse.tile_rust import add_dep_helper

    f32 = mybir.dt.float32
    bf16 = mybir.dt.bfloat16
    Alu = mybir.AluOpType
    Act = mybir.ActivationFunctionType

    nc = bacc.Bacc()
    fm = nc.dram_tensor("fm", [P, FREE], bf16, kind="ExternalInput")
    # wd1: w1 block-diag (128 cols) + per-chunk L4 weights (2 x 8 cols) +
    # the four fp32 bias columns bit-packed as 8 bf16 columns, so one DMA
    # carries everything the first matmul and crossing need.
    # wd2: w2, w3 block-diags
    W1C = P + NCHUNK * OROWS             # 144
    wd1 = nc.dram_tensor("wd1", [P, W1C + 8], bf16, kind="ExternalInput")
    wd2 = nc.dram_tensor("wd2", [P, 2 * P], bf16, kind="ExternalInput")
    out = nc.dram_tensor("out", [NPIX], f32, kind="ExternalOutput")

    # out[m*FREE + s*SCW + c*CH + n] viewed [s][c][m][n] for group DMAs
    out_s = out.rearrange("(m s c n) -> s c m n", m=RG, s=NSC, c=NCHUNK, n=CH)

    # input DMA groups: group index -> (start_sc, width)
    group_start = []
    s0 = 0
    for g in IN_GROUPS:
        group_start.append((s0, g))
        s0 += g
    sc_to_group = {}
    for gi, (s0, g) in enumerate(group_start):
        for s in range(s0, s0 + g):
            sc_to_group[s] = gi

    # L4 group structure: sc -> (group, q, group_start, group_len)
    l4_of = {}
    dma_at = {}      # tick -> [sc] whose output DMA is emitted that tick
    g0 = 0
    for gi, glen in enumerate(L4_GROUPS):
        for q in range(glen):
            l4_of[g0 + q] = (gi, q, g0, glen)
            # fin4(gi) is emitted at tick g0+glen+2; ship two sc per tick
            dma_at.setdefault(g0 + glen + 3 + q // 2, []).append(g0 + q)
        g0 += glen

    busy = {"A": 0.0, "D": 0.0}

    def cross(ret_out, ret_in, bcol, kind, relu, info):
        """Emit the PSUM->SBUF crossing on the engine with less backlog."""
        eng = "A" if busy["A"] + COST[("A", kind)] <= busy["D"] + COST[("D", kind)] else "D"
        busy[eng] += COST[(eng, kind)]
        if eng == "A":
            xop = nc.scalar.activation(
                ret_out, ret_in, Act.Relu if relu else Act.Identity,
                bias=bcol, scale=1.0,
            )
        else:
            if relu:
                xop = nc.vector.tensor_scalar(
                    ret_out, ret_in, bcol, 0.0, Alu.add, Alu.max)
            else:
                xop = nc.vector.tensor_scalar(
                    ret_out, ret_in, bcol, None, Alu.add)
        NAME_INFO[xop.ins.name] = info + (eng,)
        return xop

    with TileContext(nc) as tc:
        with (
            tc.tile_pool(name="const", bufs=1) as constp,
            tc.tile_pool(name="data", bufs=6) as datap,
            tc.tile_pool(name="acts", bufs=2) as actp,
            tc.tile_pool(name="outs", bufs=2) as outsp,
            tc.tile_pool(name="psb", bufs=1, space="PSUM") as psb,
        ):
            # --- startup: tiny gating DMAs + PE warmup ---
            group_tiles = {}
            def load_group(gi, issuer=None):
                s0, g = group_start[gi]
                xb = datap.tile([P, g * SCW], bf16, tag="x", name=f"xb{gi}")
                group_tiles[gi] = xb
                (issuer or nc.sync).dma_start(
                    xb, fm[:, s0 * SCW:(s0 + g) * SCW])

            # first input group goes through Pool's SWDGE path so its
            # descriptor generation runs in parallel with the weight DMA
            # on SP; the warmup memset rides the otherwise-idle DVE
            load_group(0, issuer=nc.gpsimd)
            w1t = constp.tile([P, W1C + 8], bf16)
            nc.sync.dma_start(w1t, wd1[:, :])
            btile = w1t[:, W1C:W1C + 8].bitcast(f32)
            scratch = constp.tile([P, CH], bf16)
            nc.vector.memset(scratch[:, :], 0.0)
            w23t = constp.tile([P, 2 * P], bf16)
            nc.sync.dma_start(w23t, wd2[:, :])

            # single 8-bank PSUM tile; subtile range deps replace the
            # tag/bufs rotation. Banks 0-6 rotate data chunks; bank 7 is
            # the stationary packed-L4 accumulator.
            PS = psb.tile([P, 8 * CH], f32, name="PS")
            l4b = PS[:, NDB * CH:(NDB + 1) * CH]
            for i in range(N_WARMUP):
                wm = nc.tensor.matmul(
                    l4b, scratch[:, :P], scratch[:, :],
                    start=True, stop=True,
                )
                NAME_INFO[wm.ins.name] = (-1, f"warm{i}", "PE")

            wsl = [
                w1t[:, :P],
                w23t[:, :P],
                w23t[:, P:2 * P],
                [w1t[:, P + cc * OROWS:P + (cc + 1) * OROWS]
                 for cc in range(NCHUNK)],
            ]

            h_of = {}        # sc -> current activation tile
            ob_of = {}       # L4 group -> evacuated SBUF tile
            fin_of = {}      # L4 group -> evac instruction (DMA gate)
            loaded = 1       # groups issued (group 0 already out)
            ctr = 0          # data-chunk counter driving the bank rotation

            for t in range(NSC + 7):
                # prefetch input groups
                while loaded < len(group_start) and group_start[loaded][0] - PREFETCH <= t:
                    load_group(loaded)
                    loaded += 1

                # L1(t), L2(t-1), L3(t-2) matmuls + crossings
                for layer in range(3):
                    s = t - layer
                    if not (0 <= s < NSC):
                        continue
                    if layer == 0:
                        gi = sc_to_group[s]
                        si = s - group_start[gi][0]
                        h = group_tiles[gi][:, si * SCW:(si + 1) * SCW]
                    else:
                        h = h_of[s]
                    # adjacent bank pair from the 7-bank rotation: reuse
                    # distance ~2 ticks >> crossing latency
                    bA, bB = ctr % NDB, (ctr + 1) % NDB
                    ctr += 2
                    banks = (bA, bB)
                    for cc in range(NCHUNK):
                        mm = nc.tensor.matmul(
                            PS[:, banks[cc] * CH:(banks[cc] + 1) * CH],
                            wsl[layer],
                            h[:, cc * CH:(cc + 1) * CH],
                            start=True, stop=True,
                        )
                        NAME_INFO[mm.ins.name] = (s, f"mm{layer}.{cc}")
                    hn = actp.tile([P, SCW], bf16, tag=f"h{layer}")
                    bcol = btile[:, layer:layer + 1]
                    if bB == bA + 1:
                        cross(hn[:, :], PS[:, bA * CH:(bB + 1) * CH],
                              bcol, "big", True, (s, f"relu{layer}"))
                    else:
                        # wrap pair: two contiguous half-crossings so the
                        # range tracker never sees a spanning bounding box
                        for cc in range(NCHUNK):
                            cross(hn[:, cc * CH:(cc + 1) * CH],
                                  PS[:, banks[cc] * CH:(banks[cc] + 1) * CH],
                                  bcol, "half", True, (s, f"relu{layer}.{cc}"))
                    h_of[s] = hn

                # one output DMA per super-chunk (contiguous 8-row source
                # slice of the group's ob), spread 1/tick after the group's
                # evac op is emitted
                # SBUF APs address partitions only via their first dim, so
                # the source must be a plain partition range: one DMA per
                # (super-chunk, chunk)
                for s in dma_at.get(t, ()):
                    g, q, gs, glen = l4_of[s]
                    ob = ob_of[g]
                    for cc in range(NCHUNK):
                        # split issue across the SP and (idle) Pool
                        # sequencers: SP's 650ns-per-DMA issue rate is the
                        # output-path bottleneck. The drain tail avoids
                        # Pool (~1us SWDGE generation would serialize it);
                        # the very last pair issues SP || Act.SEQ in
                        # parallel -- the Act engine is finished by then.
                        if s == NSC - 1:
                            issuer = nc.sync if cc == 0 else nc.scalar
                        elif s == NSC - 2:
                            issuer = nc.sync if cc == 0 else nc.scalar
                        elif s >= NSC - 3:
                            issuer = nc.sync
                        else:
                            issuer = nc.sync if cc == (s % 2) else nc.gpsimd
                        dm = issuer.dma_start(
                            out_s[gs + q, cc, :, :],
                            ob[32 * q + RG * cc:32 * q + RG * cc + RG, :],
                        )
                        add_dep_helper(dm.ins, fin_of[g], reason="fin4 gate")
                        NAME_INFO[dm.ins.name] = (s, f"outdma{cc}")

                # L4(t-3): pack 4 super-chunks' outputs into PSUM bank 7
                # at partition offsets 32q; rows 4cc+m within each block.
                # One full-partition evac op then covers the whole group.
                s = t - 3
                if 0 <= s < NSC:
                    h = h_of.pop(s)
                    g, q, gs, glen = l4_of[s]
                    for cc in range(NCHUNK):
                        mm4 = nc.tensor.matmul(
                            l4b[32 * q:32 * q + OROWS, :],
                            wsl[3][cc],
                            h[:, cc * CH:(cc + 1) * CH],
                            start=(cc == 0), stop=(cc == NCHUNK - 1),
                            tile_position=(0, 32 * q),
                        )
                        NAME_INFO[mm4.ins.name] = (s, f"mm4.{cc}")
                    if q == glen - 1:
                        ob_of[g] = outsp.tile(
                            [P, CH], f32, tag=f"ob{g}", name=f"ob{g}")
                        fop = cross(ob_of[g][:32 * glen, :], l4b[:32 * glen, :],
                                    btile[:32 * glen, 3:4],
                                    "fin4", False, (s, "final4"))
                        fin_of[g] = fop.ins



    _hoist_matmul_waits(nc, mybir)
    nc.compile()
    return nc


def _hoist_matmul_waits(nc, mybir):
    """Walrus codegen cannot reliably attach semaphore waits to self-loading
    matmuls; hoist every matmul's waits onto a PE nop inserted just before
    it (sequencer-side wait, same semantics)."""
    for blk in nc.main_func.blocks:
        insts = blk.instructions
        idx = 0
        while idx < len(insts):
            inst = insts[idx]
            if isinstance(inst, mybir.InstMatmult):
                si = inst.sync_info
                if si is not None and len(si.on_wait) > 0:
                    nop = mybir.InstNoOp(
                        name=nc.get_next_instruction_name(), ins=[], outs=[]
                    )
                    nop.engine = inst.engine
                    nop.bass_nofuse = True
                    nop.sync_info = mybir.SyncInfo(on_wait=si.on_wait, on_update=[])
                    si.on_wait = []
                    nc.register_instruction(nop)
                    insts.insert(idx, nop)
                    idx += 1
            idx += 1
    for blk in nc.main_func.blocks:
        for inst in blk.instructions:
            if isinstance(inst, mybir.InstMatmult):
                si = inst.sync_info
                assert si is None or len(si.on_wait) == 0, inst.name


def _blockdiag4(wT):
    """[32, 32] -> [128, 128] block-diagonal with 4 copies."""
    out = np.zeros((P, P), dtype=np.float32)
    for b in range(RG):
        out[32 * b:32 * b + 32, 32 * b:32 * b + 32] = wT
    return out


def _prep_host_inputs(z, w1, b1, w2, b2, w3, b3, wl, bl):
    """Fold z into the layer-1 bias and build the device weight layouts."""
    import ml_dtypes

    f32 = np.float32
    bf16 = ml_dtypes.bfloat16
    b1e = (b1 + w1[:, C:] @ z[0]).astype(f32)          # [32]

    # per-chunk L4 weights: chunk cc's [128, 8] lhsT puts block m's wl dot
    # on output row 4cc+m (other rows contribute zero and accumulate)
    w4 = np.zeros((P, NCHUNK * OROWS), dtype=f32)
    for cc in range(NCHUNK):
        for m in range(RG):
            w4[32 * m:32 * m + 32, cc * OROWS + RG * cc + m] = wl[0, :]

    bias = np.zeros((P, 4), dtype=f32)
    bias[:, 0] = np.tile(b1e, RG)
    bias[:, 1] = np.tile(b2.astype(f32), RG)
    bias[:, 2] = np.tile(b3.astype(f32), RG)
    bias[:, 3] = f32(bl[0])

    wd1 = np.concatenate(
        [
            _blockdiag4(w1[:, :C].T).astype(bf16),
            w4.astype(bf16),
            np.ascontiguousarray(bias).view(bf16),   # fp32 bits as bf16 pairs
        ],
        axis=1,
    )
    wd2 = np.concatenate([_blockdiag4(w2.T), _blockdiag4(w3.T)], axis=1).astype(bf16)
    return wd1, wd2


def _restripe(shard):
    """[32, npix] channel-major shard -> [128, npix/4] (block, channel) rows."""
    import ml_dtypes

    npix = shard.shape[1]
    return np.ascontiguousarray(
        shard.reshape(C, RG, npix // RG).transpose(1, 0, 2).reshape(P, npix // RG)
    ).astype(ml_dtypes.bfloat16)


_NC_CACHE = {}
NAME_INFO = {}   # instruction name -> (sc, stage) for profiling


def _run(feature_map, z, w1, b1, w2, b2, w3, b3, wl, bl, **spmd_kwargs):
    from concourse.bass_utils import run_bass_kernel_spmd

    feature_map = np.asarray(feature_map, dtype=np.float32)
    z = np.asarray(z, dtype=np.float32)
    w1, b1 = np.asarray(w1, np.float32), np.asarray(b1, np.float32)
    w2, b2 = np.asarray(w2, np.float32), np.asarray(b2, np.float32)
    w3, b3 = np.asarray(w3, np.float32), np.asarray(b3, np.float32)
    wl, bl = np.asarray(wl, np.float32), np.asarray(bl, np.float32)

    wd1, wd2 = _prep_host_inputs(z, w1, b1, w2, b2, w3, b3, wl, bl)

    fm_flat = feature_map.reshape(C, VOL)
    in_maps = []
    for k in range(NCORES):
        shard = _restripe(fm_flat[:, k * NPIX:(k + 1) * NPIX])
        in_maps.append({"fm": shard, "wd1": wd1, "wd2": wd2})

    if "nc" not in _NC_CACHE:
        _NC_CACHE["nc"] = _build_nc()
    nc = _NC_CACHE["nc"]

    res = run_bass_kernel_spmd(nc, in_maps, core_ids=list(range(NCORES)), **spmd_kwargs)
    out = np.empty((VOL,), dtype=np.float32)
    for k in range(NCORES):
        out[k * NPIX:(k + 1) * NPIX] = res.results[k]["out"]
    return out.reshape(1, 1, 96, 96, 96), res


def kernel(feature_map, z, w1, b1, w2, b2, w3, b3, wl, bl):
    out, _ = _run(feature_map, z, w1, b1, w2, b2, w3, b3, wl, bl)
    return out


# revision 116
# speedup vs baseline: 1.0096x; 1.0096x over previous
"""Trainium2 Bass kernel for nn_FComb_79319456023150 (dense_cnn).

Per-pixel MLP over a 96^3 volume: four 1x1x1 convs (38->32->32->32->1 channels,
relu between). z is batch-constant, so w1[:, 32:38] @ z folds into the layer-1
bias and every layer becomes a K=32 channel GEMM.

Sharding: spatial (outermost X axis) across 8 cores, 110592 pixels each;
weights/biases replicated. Host restripes each shard to [128, 27648] bf16 =
4 pixel-blocks x 32 channels on partitions, pixels on the free dim (bf16
halves input DMA bytes; rel err ~5e-3 vs the 2e-2 tolerance).

Layers 1-3 use a block-diagonal [128,128] bf16 weight (4 copies of W^T), one
matmul per 512-col PSUM bank chunk, so one full-partition matmul applies the
32x32 GEMM to 4 pixel blocks at once. Schedule: depth-4 software pipeline --
tick t emits matmuls for L1(t), L2(t-1), L3(t-2), L4(t-3) -- so each relu
crossing's ~1.3us latency hides behind three other super-chunks' matmuls.

PSUM is ONE 8-bank tile with subtile range deps. Banks 0-6 rotate data
chunks (adjacent pairs, reuse distance ~2 ticks >> crossing latency); pairs
that wrap bank 6->0 split into two half-crossings so every engine/DMA AP
keeps its partition range in the first AP dim (SBUF APs address partitions
ONLY via dim 0 -- nested partition splits silently read garbage). Bank 7 is
a stationary L4 accumulator: sc s writes rows 32q..32q+7 (q = s mod 4, via
per-chunk [128,8] lhsT + tile_position=(0,32q)), so ONE [128,512] evac op
covers 4 super-chunks -- cutting the L4 evacuation engine work ~2.5x vs
per-sc evacuation.

The relu+bias crossings (fp32 PSUM reads run 1 elem/cycle/lane; GPSIMD has
no PSUM port) are greedily balanced between ScalarE (1.2 GHz) and VectorE
(0.96 GHz) by accumulated busy time -- these two engines are the throughput
bound (~49.5us each). Output DMAs (2 per sc, [4,512] plain partition
slices) split their ~0.65us/DMA issue cost between the SP and idle Pool
sequencers; the drain tail is SP-only. Startup: the first input super-chunk
rides Pool's SWDGE in parallel with the w1+L4+bias-bits DMA on SP, and
dummy matmuls on a memset scratch tile burn the PE p-state ramp during the
DMA wait. 64160 ns vs the 71309 ns baseline (cost-model timeline).
"""

import sys

import numpy as np

if "/opt/trn_rl_repo" not in sys.path:
    sys.path.insert(0, "/opt/trn_rl_repo")

C = 32          # channels per layer
P = 128         # SBUF/PSUM partitions
RG = 4          # pixel blocks stacked on the partition dim (128/32)
NCHUNK = 2      # 512-wide chunks per super-chunk (PSUM slot = 2 banks)
CH = 512        # chunk width (one PSUM bank of fp32)
SCW = NCHUNK * CH                    # 1024 free-dim columns per super-chunk
VOL = 96 * 96 * 96                   # full volume
NCORES = 8
NPIX = VOL // NCORES                 # 110592 pixels per core
FREE = NPIX // RG                    # 27648 free-dim columns per core
NSC = FREE // SCW                    # 27 super-chunks per core
OROWS = RG * NCHUNK                  # 8 packed output rows per super-chunk
assert FREE % SCW == 0

N_WARMUP = 3                         # dummy PE matmuls during startup DMA
IN_GROUPS = [1, 1, 1] + [3] * 8      # super-chunks per input DMA
PREFETCH = 5                         # emit input DMA this many ticks early
L4_GROUPS = [4] * 6 + [1, 1, 1]      # super-chunks per packed L4 bank epoch
assert sum(IN_GROUPS) == NSC and sum(L4_GROUPS) == NSC

# engine busy-time estimates (ns) for the greedy balancer. A fin4 op is
# [128, 512]: engine time depends only on the 512-col free size, so it
# costs the same as a half crossing.
COST = {
    ("A", "big"): 1038.0, ("D", "big"): 1192.0,
    ("A", "fin4"): 612.0, ("D", "fin4"): 658.0,
    ("A", "half"): 612.0, ("D", "half"): 658.0,
}
L4GROUP = 4                          # super-chunks packed per L4 PSUM bank
NDB = 7                              # data banks in the chunk rotation


def _build_nc():
    import concourse.mybir as mybir
    from concourse import bacc
    from concourse.tile import TileContext
    from concourse.tile_rust import add_dep_helper

    f32 = mybir.dt.float32
    bf16 = mybir.dt.bfloat16
    Alu = mybir.AluOpType
    Act = mybir.ActivationFunctionType

    nc = bacc.Bacc()
    fm = nc.dram_tensor("fm", [P, FREE], bf16, kind="ExternalInput")
    # wd1: w1 block-diag (128 cols) + per-chunk L4 weights (2 x 8 cols) +
    # the four fp32 bias columns bit-packed as 8 bf16 columns, so one DMA
    # carries everything the first matmul and crossing need.
    # wd2: w2, w3 block-diags
    W1C = P + NCHUNK * OROWS             # 144
    wd1 = nc.dram_tensor("wd1", [P, W1C + 8], bf16, kind="ExternalInput")
    wd2 = nc.dram_tensor("wd2", [P, 2 * P], bf16, kind="ExternalInput")
    out = nc.dram_tensor("out", [NPIX], f32, kind="ExternalOutput")

    # out[m*FREE + s*SCW + c*CH + n] viewed [s][c][m][n] for group DMAs
    out_s = out.rearrange("(m s c n) -> s c m n", m=RG, s=NSC, c=NCHUNK, n=CH)

    # input DMA groups: group index -> (start_sc, width)
    group_start = []
    s0 = 0
    for g in IN_GROUPS:
        group_start.append((s0, g))
        s0 += g
    sc_to_group = {}
    for gi, (s0, g) in enumerate(group_start):
        for s in range(s0, s0 + g):
            sc_to_group[s] = gi

    # L4 group structure: sc -> (group, q, group_start, group_len)
    l4_of = {}
    dma_at = {}      # tick -> [sc] whose output DMA is emitted that tick
    g0 = 0
    for gi, glen in enumerate(L4_GROUPS):
        for q in range(glen):
            l4_of[g0 + q] = (gi, q, g0, glen)
            # fin4(gi) is emitted at tick g0+glen+2; ship two sc per tick
            dma_at.setdefault(g0 + glen + 3 + q // 2, []).append(g0 + q)
        g0 += glen

    busy = {"A": 0.0, "D": 0.0}
    last_eng = ["D"]

    def cross(ret_out, ret_in, bcol, kind, relu, info):
        """Emit the PSUM->SBUF crossing on the engine with less backlog,
        preferring to alternate engines on consecutive ops (same-engine
        runs serialize behind head-of-line waits while the other starves)."""
        pref = "D" if last_eng[0] == "A" else "A"
        other = "D" if pref == "A" else "A"
        if busy[pref] + COST[(pref, kind)] <= busy[other] + COST[(other, kind)] + 400.0:
            eng = pref
        else:
            eng = other
        last_eng[0] = eng
        busy[eng] += COST[(eng, kind)]
        if eng == "A":
            xop = nc.scalar.activation(
                ret_out, ret_in, Act.Relu if relu else Act.Identity,
                bias=bcol, scale=1.0,
            )
        else:
            if relu:
                xop = nc.vector.tensor_scalar(
                    ret_out, ret_in, bcol, 0.0, Alu.add, Alu.max)
            else:
                xop = nc.vector.tensor_scalar(
                    ret_out, ret_in, bcol, None, Alu.add)
        NAME_INFO[xop.ins.name] = info + (eng,)
        return xop

    with TileContext(nc) as tc:
        with (
            tc.tile_pool(name="const", bufs=1) as constp,
            tc.tile_pool(name="data", bufs=6) as datap,
            tc.tile_pool(name="acts", bufs=2) as actp,
            tc.tile_pool(name="outs", bufs=2) as outsp,
            tc.tile_pool(name="psb", bufs=1, space="PSUM") as psb,
        ):
            # --- startup: tiny gating DMAs + PE warmup ---
            group_tiles = {}
            def load_group(gi, issuer=None):
                s0, g = group_start[gi]
                xb = datap.tile([P, g * SCW], bf16, tag="x", name=f"xb{gi}")
                group_tiles[gi] = xb
                (issuer or nc.sync).dma_start(
                    xb, fm[:, s0 * SCW:(s0 + g) * SCW])

            # first input group goes through Pool's SWDGE path so its
            # descriptor generation runs in parallel with the weight DMA
            # on SP; the warmup memset rides the otherwise-idle DVE
            load_group(0, issuer=nc.gpsimd)
            w1t = constp.tile([P, W1C + 8], bf16)
            nc.sync.dma_start(w1t, wd1[:, :])
            btile = w1t[:, W1C:W1C + 8].bitcast(f32)
            scratch = constp.tile([P, CH], bf16)
            nc.vector.memset(scratch[:, :], 0.0)
            w23t = constp.tile([P, 2 * P], bf16)
            nc.sync.dma_start(w23t, wd2[:, :])

            # single 8-bank PSUM tile; subtile range deps replace the
            # tag/bufs rotation. Banks 0-6 rotate data chunks; bank 7 is
            # the stationary packed-L4 accumulator.
            PS = psb.tile([P, 8 * CH], f32, name="PS")
            l4b = PS[:, NDB * CH:(NDB + 1) * CH]
            for i in range(N_WARMUP):
                wm = nc.tensor.matmul(
                    l4b, scratch[:, :P], scratch[:, :],
                    start=True, stop=True,
                )
                NAME_INFO[wm.ins.name] = (-1, f"warm{i}", "PE")

            wsl = [
                w1t[:, :P],
                w23t[:, :P],
                w23t[:, P:2 * P],
                [w1t[:, P + cc * OROWS:P + (cc + 1) * OROWS]
                 for cc in range(NCHUNK)],
            ]

            h_of = {}        # sc -> current activation tile
            ob_of = {}       # L4 group -> evacuated SBUF tile
            fin_of = {}      # L4 group -> evac instruction (DMA gate)
            loaded = 1       # groups issued (group 0 already out)
            ctr = 0          # data-chunk counter driving the bank rotation

            for t in range(NSC + 7):
                # prefetch input groups
                while loaded < len(group_start) and group_start[loaded][0] - PREFETCH <= t:
                    load_group(loaded)
                    loaded += 1

                # L1(t), L2(t-1), L3(t-2) matmuls + crossings
                for layer in range(3):
                    s = t - layer
                    if not (0 <= s < NSC):
                        continue
                    if layer == 0:
                        gi = sc_to_group[s]
                        si = s - group_start[gi][0]
                        h = group_tiles[gi][:, si * SCW:(si + 1) * SCW]
                    else:
                        h = h_of[s]
                    # adjacent bank pair from the 7-bank rotation: reuse
                    # distance ~2 ticks >> crossing latency
                    bA, bB = ctr % NDB, (ctr + 1) % NDB
                    ctr += 2
                    banks = (bA, bB)
                    for cc in range(NCHUNK):
                        mm = nc.tensor.matmul(
                            PS[:, banks[cc] * CH:(banks[cc] + 1) * CH],
                            wsl[layer],
                            h[:, cc * CH:(cc + 1) * CH],
                            start=True, stop=True,
                        )
                        NAME_INFO[mm.ins.name] = (s, f"mm{layer}.{cc}")
                    hn = actp.tile([P, SCW], bf16, tag=f"h{layer}")
                    bcol = btile[:, layer:layer + 1]
                    if bB == bA + 1:
                        cross(hn[:, :], PS[:, bA * CH:(bB + 1) * CH],
                              bcol, "big", True, (s, f"relu{layer}"))
                    else:
                        # wrap pair: two contiguous half-crossings so the
                        # range tracker never sees a spanning bounding box
                        for cc in range(NCHUNK):
                            cross(hn[:, cc * CH:(cc + 1) * CH],
                                  PS[:, banks[cc] * CH:(banks[cc] + 1) * CH],
                                  bcol, "half", True, (s, f"relu{layer}.{cc}"))
                    h_of[s] = hn

                # one output DMA per super-chunk (contiguous 8-row source
                # slice of the group's ob), spread 1/tick after the group's
                # evac op is emitted
                # SBUF APs address partitions only via their first dim, so
                # the source must be a plain partition range: one DMA per
                # (super-chunk, chunk)
                for s in dma_at.get(t, ()):
                    g, q, gs, glen = l4_of[s]
                    ob = ob_of[g]
                    for cc in range(NCHUNK):
                        # split issue across the SP and (idle) Pool
                        # sequencers: SP's 650ns-per-DMA issue rate is the
                        # output-path bottleneck. The drain tail avoids
                        # Pool (~1us SWDGE generation would serialize it);
                        # the very last pair issues SP || Act.SEQ in
                        # parallel -- the Act engine is finished by then.
                        if s == NSC - 1:
                            issuer = nc.sync if cc == 0 else nc.scalar
                        elif s == NSC - 2:
                            issuer = nc.sync if cc == 0 else nc.scalar
                        elif s >= NSC - 3:
                            issuer = nc.sync
                        else:
                            issuer = nc.sync if cc == (s % 2) else nc.gpsimd
                        dm = issuer.dma_start(
                            out_s[gs + q, cc, :, :],
                            ob[32 * q + RG * cc:32 * q + RG * cc + RG, :],
                        )
                        add_dep_helper(dm.ins, fin_of[g], reason="fin4 gate")
                        NAME_INFO[dm.ins.name] = (s, f"outdma{cc}")

                # L4(t-3): pack 4 super-chunks' outputs into PSUM bank 7
                # at partition offsets 32q; rows 4cc+m within each block.
                # One full-partition evac op then covers the whole group.
                s = t - 3
                if 0 <= s < NSC:
                    h = h_of.pop(s)
                    g, q, gs, glen = l4_of[s]
                    for cc in range(NCHUNK):
                        mm4 = nc.tensor.matmul(
                            l4b[32 * q:32 * q + OROWS, :],
                            wsl[3][cc],
                            h[:, cc * CH:(cc + 1) * CH],
                            start=(cc == 0), stop=(cc == NCHUNK - 1),
                            tile_position=(0, 32 * q),
                        )
                        NAME_INFO[mm4.ins.name] = (s, f"mm4.{cc}")
                    if q == glen - 1:
                        ob_of[g] = outsp.tile(
                            [P, CH], f32, tag=f"ob{g}", name=f"ob{g}")
                        fop = cross(ob_of[g][:32 * glen, :], l4b[:32 * glen, :],
                                    btile[:32 * glen, 3:4],
                                    "fin4", False, (s, "final4"))
                        fin_of[g] = fop.ins



    _hoist_matmul_waits(nc, mybir)
    nc.compile()
    return nc


def _hoist_matmul_waits(nc, mybir):
    """Walrus codegen cannot reliably attach semaphore waits to self-loading
    matmuls; hoist every matmul's waits onto a PE nop inserted just before
    it (sequencer-side wait, same semantics)."""
    for blk in nc.main_func.blocks:
        insts = blk.instructions
        idx = 0
        while idx < len(insts):
            inst = insts[idx]
            if isinstance(inst, mybir.InstMatmult):
                si = inst.sync_info
                if si is not None and len(si.on_wait) > 0:
                    nop = mybir.InstNoOp(
                        name=nc.get_next_instruction_name(), ins=[], outs=[]
                    )
                    nop.engine = inst.engine
                    nop.bass_nofuse = True
                    nop.sync_info = mybir.SyncInfo(on_wait=si.on_wait, on_update=[])
                    si.on_wait = []
                    nc.register_instruction(nop)
                    insts.insert(idx, nop)
                    idx += 1
            idx += 1
    for blk in nc.main_func.blocks:
        for inst in blk.instructions:
            if isinstance(inst, mybir.InstMatmult):
                si = inst.sync_info
                assert si is None or len(si.on_wait) == 0, inst.name


def _blockdiag4(wT):
    """[32, 32] -> [128, 128] block-diagonal with 4 copies."""
    out = np.zeros((P, P), dtype=np.float32)
    for b in range(RG):
        out[32 * b:32 * b + 32, 32 * b:32 * b + 32] = wT
    return out


def _prep_host_inputs(z, w1, b1, w2, b2, w3, b3, wl, bl):
    """Fold z into the layer-1 bias and build the device weight layouts."""
    import ml_dtypes

    f32 = np.float32
    bf16 = ml_dtypes.bfloat16
    b1e = (b1 + w1[:, C:] @ z[0]).astype(f32)          # [32]

    # per-chunk L4 weights: chunk cc's [128, 8] lhsT puts block m's wl dot
    # on output row 4cc+m (other rows contribute zero and accumulate)
    w4 = np.zeros((P, NCHUNK * OROWS), dtype=f32)
    for cc in range(NCHUNK):
        for m in range(RG):
            w4[32 * m:32 * m + 32, cc * OROWS + RG * cc + m] = wl[0, :]

    bias = np.zeros((P, 4), dtype=f32)
    bias[:, 0] = np.tile(b1e, RG)
    bias[:, 1] = np.tile(b2.astype(f32), RG)
    bias[:, 2] = np.tile(b3.astype(f32), RG)
    bias[:, 3] = f32(bl[0])

    wd1 = np.concatenate(
        [
            _blockdiag4(w1[:, :C].T).astype(bf16),
            w4.astype(bf16),
            np.ascontiguousarray(bias).view(bf16),   # fp32 bits as bf16 pairs
        ],
        axis=1,
    )
    wd2 = np.concatenate([_blockdiag4(w2.T), _blockdiag4(w3.T)], axis=1).astype(bf16)
    return wd1, wd2


def _restripe(shard):
    """[32, npix] channel-major shard -> [128, npix/4] (block, channel) rows."""
    import ml_dtypes

    npix = shard.shape[1]
    return np.ascontiguousarray(
        shard.reshape(C, RG, npix // RG).transpose(1, 0, 2).reshape(P, npix // RG)
    ).astype(ml_dtypes.bfloat16)


_NC_CACHE = {}
NAME_INFO = {}   # instruction name -> (sc, stage) for profiling


def _run(feature_map, z, w1, b1, w2, b2, w3, b3, wl, bl, **spmd_kwargs):
    from concourse.bass_utils import run_bass_kernel_spmd

    feature_map = np.asarray(feature_map, dtype=np.float32)
    z = np.asarray(z, dtype=np.float32)
    w1, b1 = np.asarray(w1, np.float32), np.asarray(b1, np.float32)
    w2, b2 = np.asarray(w2, np.float32), np.asarray(b2, np.float32)
    w3, b3 = np.asarray(w3, np.float32), np.asarray(b3, np.float32)
    wl, bl = np.asarray(wl, np.float32), np.asarray(bl, np.float32)

    wd1, wd2 = _prep_host_inputs(z, w1, b1, w2, b2, w3, b3, wl, bl)

    fm_flat = feature_map.reshape(C, VOL)
    in_maps = []
    for k in range(NCORES):
        shard = _restripe(fm_flat[:, k * NPIX:(k + 1) * NPIX])
        in_maps.append({"fm": shard, "wd1": wd1, "wd2": wd2})

    if "nc" not in _NC_CACHE:
        _NC_CACHE["nc"] = _build_nc()
    nc = _NC_CACHE["nc"]

    res = run_bass_kernel_spmd(nc, in_maps, core_ids=list(range(NCORES)), **spmd_kwargs)
    out = np.empty((VOL,), dtype=np.float32)
    for k in range(NCORES):
        out[k * NPIX:(k + 1) * NPIX] = res.results[k]["out"]
    return out.reshape(1, 1, 96, 96, 96), res


def kernel(feature_map, z, w1, b1, w2, b2, w3, b3, wl, bl):
    out, _ = _run(feature_map, z, w1, b1, w2, b2, w3, b3, wl, bl)
    return out
